# revision 7
# baseline (speedup 1.0000x reference)
"""Trainium2 Bass kernel: CrossframeLocalInterpolationModule (gnn message passing).

Computation per vertex n (N=500000, C=32, K=9):
  neigh  = hidden_state[safe_idx]                (masked gather)
  dist_k = ||neigh_k - lv_n||_2 * valid_k
  dist_n = dist / sum_k dist
  w_k    = relu(alpha - dist_n) * beta * valid_k
  aflow  = sum_k w_k * neigh_k + b_aflow
  out    = relu([aflow, lv] @ W + b_lin)

Bottleneck analysis: the neighbor gather must use per-slot indirect DMAs
(128 rows / instruction, ~1.4us SWDGE generation cadence on the Pool
engine, payload-size independent). So the only real lever is fewer gather
instructions. Host-side prep:
  - vertices sorted by descending valid-neighbor count, each vertex's
    valid slots compacted to the front => per 128-vertex subtile only
    max-valid-count gather instructions (~15% fewer instructions and
    bytes). Output rows are inverse-permuted on the host.
  - indices pre-clamped, and the (valid * beta) mask precomputed on host.
  - hidden_state / lv / weights converted to fp16 on host: halves gather
    payload + table upload and doubles DVE/PE throughput for the math.

Identity used on-device: sum_k w_k*neigh_k = sum_k w_k*(neigh_k - lv) + (sum_k w_k)*lv
so the gathered tile can be destroyed in-place by the diff computation.
"""

import numpy as np

try:
    import concourse.bass as bass
except ImportError:  # pragma: no cover - fallback path
    import sys

    sys.path.insert(0, "/opt/trn_rl_repo")
    import concourse.bass as bass

import concourse.bacc as bacc

from contextlib import ExitStack

import concourse.tile as tile_mod
from concourse import mybir
from concourse.bass_utils import run_bass_kernel_spmd
from concourse.masks import make_identity

F32 = mybir.dt.float32
F16 = mybir.dt.float16
I32 = mybir.dt.int32
ALU = mybir.AluOpType
ACTF = mybir.ActivationFunctionType
AX = mybir.AxisListType

N_FULL = 500000
C = 32
K = 9
NCORES = 8
P = 128
T_MAIN = 8  # 128-vertex sub-tiles per big tile

# pad so every core gets an equal whole number of 128-vertex sub-tiles
PER_CORE = 62592  # = 489 * 128 ;  8 * 62592 = 500736 >= 500000
PAD_N = PER_CORE * NCORES


def _subtile_plan(per_core, t_main):
    s = per_core // P
    tiles = [t_main] * (s // t_main)
    if s % t_main:
        tiles.append(s % t_main)
    return tiles


def _ap(base, dims):
    """Build an AP on the same tensor as `base` ([P, free...] tile view) with
    custom free dims [[step, count], ...] (element units)."""
    return bass.AP(
        tensor=base.tensor,
        offset=base.offset,
        ap=[list(base.ap[0])] + [list(d) for d in dims],
    )


def build_program(per_core, table_rows, alpha, m_per_subtile, t_main=T_MAIN):
    nc = bacc.Bacc()

    lv_d = nc.dram_tensor("lv", [per_core, C], F16, kind="ExternalInput")
    hs_d = nc.dram_tensor("hs", [table_rows, C], F16, kind="ExternalInput")
    idx_d = nc.dram_tensor("nidx", [per_core, K], I32, kind="ExternalInput")
    vmb_d = nc.dram_tensor("vmb", [per_core, K], F32, kind="ExternalInput")
    # rows 0:64 = W, row 64 = b_lin  (bias via ones-column trick)
    wb_d = nc.dram_tensor("wb", [2 * C + 1, C], F16, kind="ExternalInput")
    out_d = nc.dram_tensor("out", [per_core, C], F32, kind="ExternalOutput")

    tiles = _subtile_plan(per_core, t_main)

    with ExitStack() as ctx:
        tc = ctx.enter_context(tile_mod.TileContext(nc))
        singles = ctx.enter_context(tc.tile_pool(name="singles", bufs=1))
        ident = singles.tile([P, P], F16)
        make_identity(nc, ident[:])
        wb_sb = singles.tile([2 * C + 1, C], F16)
        nc.sync.dma_start(out=wb_sb[:], in_=wb_d[:, :])
        alpha_t = singles.tile([P, 1], F32)
        nc.vector.memset(alpha_t[:], float(alpha))

        gpool = ctx.enter_context(tc.tile_pool(name="gpool", bufs=6))
        sqpool = ctx.enter_context(tc.tile_pool(name="sqpool", bufs=3))
        catpool = ctx.enter_context(tc.tile_pool(name="catpool", bufs=4))
        idxpool = ctx.enter_context(tc.tile_pool(name="idxpool", bufs=6))
        statpool = ctx.enter_context(tc.tile_pool(name="statpool", bufs=3))
        outpool = ctx.enter_context(tc.tile_pool(name="outpool", bufs=2))
        ctpool = ctx.enter_context(tc.tile_pool(name="ctpool", bufs=3))
        tps = ctx.enter_context(tc.tile_pool(name="tps", bufs=2, space="PSUM"))
        mps = ctx.enter_context(tc.tile_pool(name="mps", bufs=2, space="PSUM"))

        # pre-zero both gather buffers once: slots skipped by the compacted
        # gather keep stale SBUF bytes, which must stay finite (0 * NaN = NaN)
        gz = []
        for _ in range(6):
            g = gpool.tile([P, t_main * K * C], F16, tag="gbuf")
            nc.vector.memset(g[:], 0.0)
            gz.append(g)
        del gz

        base = 0
        s0 = 0
        for T in tiles:
            _emit_tile(
                nc,
                pools=dict(
                    gpool=gpool,
                    sqpool=sqpool,
                    catpool=catpool,
                    idxpool=idxpool,
                    statpool=statpool,
                    outpool=outpool,
                    ctpool=ctpool,
                    tps=tps,
                    mps=mps,
                ),
                ident=ident,
                wb_sb=wb_sb,
                alpha_t=alpha_t,
                lv_d=lv_d,
                hs_d=hs_d,
                idx_d=idx_d,
                vmb_d=vmb_d,
                out_d=out_d,
                base=base,
                T=T,
                m_list=m_per_subtile[s0 : s0 + T],
                t_main=t_main,
            )
            base += T * P
            s0 += T

    nc.compile()
    return nc


def _emit_tile(nc, pools, ident, wb_sb, alpha_t, lv_d, hs_d, idx_d, vmb_d, out_d, base, T, m_list, t_main):
    KT = T * K
    F = T * K * C
    rows = T * P
    CAT = 2 * C + 1  # 65

    gpool = pools["gpool"]
    sqpool = pools["sqpool"]
    catpool = pools["catpool"]
    idxpool = pools["idxpool"]
    statpool = pools["statpool"]
    outpool = pools["outpool"]
    ctpool = pools["ctpool"]
    tps = pools["tps"]
    mps = pools["mps"]

    # vertex mapping within the tile: v = base + p * T + t
    # ---- load neighbor indices (pre-clamped, valid-compacted on host) ----
    idx_sb = idxpool.tile([P, t_main * K], I32, tag="idx")
    nc.scalar.dma_start(
        out=idx_sb[:, :KT],
        in_=idx_d[base : base + rows, :].rearrange("(p t) k -> p (t k)", t=T),
    )
    # (valid mask * beta), precomputed on host
    vmb = idxpool.tile([P, t_main * K], F32, tag="vmb")
    nc.scalar.dma_start(
        out=vmb[:, :KT],
        in_=vmb_d[base : base + rows, :].rearrange("(p t) k -> p (t k)", t=T),
    )

    # ---- gather neighbors ----
    # HW indirect DMA semantics: one descriptor per partition, one index per
    # partition (idx[p, 0]) transferring the whole per-partition out row
    # contiguously. Only the first m_list[t] compacted slots are real.
    gbuf = gpool.tile([P, t_main * K * C], F16, tag="gbuf")
    for t in range(T):
        for k in range(m_list[t]):
            j = t * K + k
            nc.gpsimd.indirect_dma_start(
                out=gbuf[:, j * C : (j + 1) * C],
                out_offset=None,
                in_=hs_d[:, :],
                in_offset=bass.IndirectOffsetOnAxis(ap=idx_sb[:, j : j + 1], axis=0),
            )

    # ---- lv load (contiguous per partition) + cat tile ----
    lvb = catpool.tile([P, t_main * C], F16, tag="lvb")
    nc.scalar.dma_start(
        out=lvb[:, : T * C],
        in_=lv_d[base : base + rows, :].rearrange("(p t) c -> p (t c)", t=T),
    )
    cat = catpool.tile([P, t_main, 2 * C + 3], F16, tag="cat")
    catw = 2 * C + 3
    nc.scalar.copy(
        out=cat[:, :T, C : 2 * C],
        in_=lvb[:, : T * C].rearrange("p (t c) -> p t c", t=T),
    )
    nc.vector.memset(cat[:, :T, 2 * C : 2 * C + 1], 1.0)

    # ---- diff = neigh - lv (lv broadcast read from cat so lvb stays ACT-only) ----
    g4 = gbuf[:, :F].rearrange("p (t k c) -> p t k c", t=T, k=K)
    dbuf = gpool.tile([P, t_main * K * C], F16, tag="dbuf")
    d4 = dbuf[:, :F].rearrange("p (t k c) -> p t k c", t=T, k=K)
    cat_base = cat[:, :, :]
    lv_bc = bass.AP(
        tensor=cat_base.tensor,
        offset=cat_base.offset + C,
        ap=[list(cat_base.ap[0]), [catw, T], [0, K], [1, C]],
    )
    nc.vector.tensor_tensor(out=d4, in0=g4, in1=lv_bc, op=ALU.subtract)

    # ---- squared distance ----
    sq = sqpool.tile([P, t_main * K * C], F16, tag="sq")
    nc.scalar.square(sq[:, :F], dbuf[:, :F])
    dsq = statpool.tile([P, t_main * K], F32, tag="dsq")
    nc.vector.tensor_reduce(
        out=dsq[:, :KT],
        in_=sq[:, :F].rearrange("p (tk c) -> p tk c", c=C),
        axis=AX.X,
        op=ALU.add,
    )
    dist = statpool.tile([P, t_main * K], F32, tag="dist")
    nc.scalar.sqrt(dist[:, :KT], dsq[:, :KT])

    # ---- masked dist, -sum, -1/sum ----
    mdist = statpool.tile([P, t_main * K], F32, tag="mdist")
    nc.vector.tensor_mul(mdist[:, :KT], dist[:, :KT], vmb[:, :KT])
    nssum = statpool.tile([P, t_main], F32, tag="nssum")
    nc.vector.tensor_reduce(
        out=nssum[:, :T],
        in_=mdist[:, :KT].rearrange("p (t k) -> p t k", k=K),
        axis=AX.X,
        op=ALU.add,
        negate=True,
    )
    nrecip = statpool.tile([P, t_main], F32, tag="nrecip")
    nc.vector.reciprocal(nrecip[:, :T], nssum[:, :T])

    # ---- w = relu(alpha - mdist/S) * vmb ----
    w = statpool.tile([P, t_main * K], F32, tag="w")
    for t in range(T):
        nc.scalar.activation(
            out=w[:, t * K : (t + 1) * K],
            in_=mdist[:, t * K : (t + 1) * K],
            func=ACTF.Relu,
            bias=alpha_t[:, :],
            scale=nrecip[:, t : t + 1],
        )
    wq = statpool.tile([P, t_main * K], F16, tag="wq")
    nc.vector.tensor_mul(wq[:, :KT], w[:, :KT], vmb[:, :KT])

    wsum = statpool.tile([P, t_main], F32, tag="wsum")
    nc.vector.tensor_reduce(
        out=wsum[:, :T],
        in_=wq[:, :KT].rearrange("p (t k) -> p t k", k=K),
        axis=AX.X,
        op=ALU.add,
    )

    # ---- wdiff = diff * w (in place), reduce over k ----
    w_bc = wq[:, :KT].rearrange("p (t k) -> p t k", k=K).to_broadcast((P, T, K, C))
    nc.vector.tensor_tensor(out=d4, in0=d4, in1=w_bc, op=ALU.mult)
    wdsum = statpool.tile([P, t_main * C], F32, tag="wdsum")
    g_kred = _ap(dbuf[:], [[K * C, T], [1, C], [C, K]])
    nc.vector.tensor_reduce(out=wdsum[:, : T * C], in_=g_kred, axis=AX.X, op=ALU.add)

    # ---- aflow = wdsum + wsum * lv  -> cat[:, t, 0:C] ----
    for t in range(T):
        nc.vector.scalar_tensor_tensor(
            out=cat[:, t, 0:C],
            in0=cat[:, t, C : 2 * C],
            scalar=wsum[:, t : t + 1],
            in1=wdsum[:, t * C : (t + 1) * C],
            op0=ALU.mult,
            op1=ALU.add,
        )

    # ---- linear layer + relu per sub-tile ----
    outsb = outpool.tile([P, t_main * C], F32, tag="outsb")
    for t in range(T):
        ctps = tps.tile([CAT, P], F16, tag="ctps")
        nc.tensor.transpose(out=ctps[:], in_=cat[:, t, 0:CAT], identity=ident[:])
        ctsb = ctpool.tile([CAT, P], F16, tag="ctsb")
        nc.scalar.copy(ctsb[:], ctps[:])
        ops = mps.tile([P, C], F32, tag="ops")
        nc.tensor.matmul(out=ops[:], lhsT=ctsb[:], rhs=wb_sb[:], start=True, stop=True)
        nc.scalar.activation(out=outsb[:, t * C : (t + 1) * C], in_=ops[:], func=ACTF.Relu)

    nc.scalar.dma_start(
        out=out_d[base : base + rows, :].rearrange("(p t) c -> p (t c)", t=T),
        in_=outsb[:, : T * C],
    )


_PROGRAM_CACHE = {}


def _get_program(per_core, table_rows, alpha, m_per_subtile, t_main=T_MAIN):
    key = (per_core, table_rows, float(alpha), tuple(m_per_subtile), t_main)
    if key not in _PROGRAM_CACHE:
        _PROGRAM_CACHE[key] = build_program(
            per_core, table_rows, alpha, list(m_per_subtile), t_main
        )
    return _PROGRAM_CACHE[key]


def _sorted_layout(neighbor_idx):
    """Host-side prep: sort vertices by descending valid count, compact valid
    slots to the front of each row, and produce the device layout order.

    Returns (dev_order, m_per_subtile_per_core) where dev_order[d] = original
    vertex index placed at device position d (device position d = core *
    PER_CORE + base + p * T + t reading sorted position base + t * 128 + p).
    """
    idx = np.asarray(neighbor_idx)
    n = idx.shape[0]
    c = (idx >= 0).sum(1).astype(np.int64)
    c_p = np.zeros(PAD_N, np.int64)
    c_p[:n] = c
    order = np.argsort(-c_p, kind="stable")  # sorted position -> vertex id

    # deal sorted vertices round-robin across cores so the per-core count
    # profiles (and thus gather-instruction loads) stay balanced
    order_by_core = order.reshape(PER_CORE, NCORES)  # [local_pos, core]
    c_by_core = c_p[order].reshape(PER_CORE, NCORES)

    # sorted position s (within a core) -> device position base + p*T + t
    tiles = _subtile_plan(PER_CORE, T_MAIN)
    dev_order = np.empty(PAD_N, np.int64)
    m_all = []
    for core in range(NCORES):
        o_core = order_by_core[:, core]
        c_core = c_by_core[:, core]
        m_core = []
        dv = np.empty(PER_CORE, np.int64)
        basep = 0
        for T in tiles:
            blk = o_core[basep : basep + T * P].reshape(T, P)
            # device pos base + p*T + t <- sorted base + t*128 + p
            dv[basep : basep + T * P] = blk.T.reshape(-1)
            cblk = c_core[basep : basep + T * P].reshape(T, P)
            m_core.extend(int(x) for x in cblk.max(1))
            basep += T * P
        dev_order[core * PER_CORE : (core + 1) * PER_CORE] = dv
        m_all.append(m_core)
    return dev_order, m_all


def _shard_inputs(lv, hidden_state, W, b_lin, b_aflow, alpha, beta, neighbor_idx):
    """Pad + sort + compact + shard on host. Returns (in_maps, dev_order, m_all)."""
    lv = np.ascontiguousarray(np.asarray(lv, dtype=np.float32))
    hs = np.ascontiguousarray(np.asarray(hidden_state, dtype=np.float32))
    idx = np.ascontiguousarray(np.asarray(neighbor_idx, dtype=np.int32))
    W = np.asarray(W, dtype=np.float32)
    b_lin = np.asarray(b_lin, dtype=np.float32)
    b_aflow = np.asarray(b_aflow, dtype=np.float32)

    n = lv.shape[0]
    dev_order, m_all = _sorted_layout(idx)

    pad = PAD_N - n
    lv_p = np.concatenate([lv, np.zeros((pad, C), np.float32)], axis=0)
    idx_p = np.concatenate([idx, np.full((pad, K), -1, np.int32)], axis=0)

    lv_dev = lv_p[dev_order].astype(np.float16)
    idx_dev = idx_p[dev_order]
    valid_dev = idx_dev >= 0
    # compact valid slots to the front of each row (stable)
    perm = np.argsort(~valid_dev, axis=1, kind="stable")
    idx_c = np.take_along_axis(idx_dev, perm, 1)
    vmb_dev = (idx_c >= 0).astype(np.float32) * np.float32(beta)
    idx_c = np.maximum(idx_c, 0).astype(np.int32)
    idx_c = np.ascontiguousarray(idx_c)
    vmb_dev = np.ascontiguousarray(vmb_dev)

    hs16 = hs.astype(np.float16)

    # fold b_aflow into the linear layer: aflow' = aflow_nobias, and
    # cat @ W + b_lin == [aflow', lv, 1] @ [[W],[b_lin + b_aflow @ W_a]]
    bias_row = b_lin + b_aflow @ W[:C, :]
    wb = np.concatenate([W, bias_row[None, :]], axis=0).astype(np.float16)

    in_maps = []
    for i in range(NCORES):
        s = i * PER_CORE
        e = s + PER_CORE
        in_maps.append(
            {
                "lv": lv_dev[s:e],
                "hs": hs16,
                "nidx": idx_c[s:e],
                "vmb": vmb_dev[s:e],
                "wb": wb,
            }
        )
    return in_maps, dev_order, m_all


def kernel(lv, hidden_state, W, b_lin, b_aflow, alpha, beta, neighbor_idx):
    n = np.asarray(lv).shape[0]
    in_maps, dev_order, m_all = _shard_inputs(
        lv, hidden_state, W, b_lin, b_aflow, alpha, beta, neighbor_idx
    )
    # all cores run the same program: use the elementwise max over cores so
    # every core gathers at least its required slots
    m_np = np.asarray(m_all, np.int64).max(0)
    nc = _get_program(
        PER_CORE, np.asarray(hidden_state).shape[0], float(alpha), tuple(int(x) for x in m_np)
    )
    res = run_bass_kernel_spmd(nc, in_maps, core_ids=list(range(NCORES)))
    out_dev = np.concatenate([res.results[i]["out"] for i in range(NCORES)], axis=0)
    out = np.empty((PAD_N, C), np.float32)
    out[dev_order] = out_dev
    return out[:n]
